# revision 41
# baseline (speedup 1.0000x reference)
"""Bass/Tile kernel for nn_EncoderBlock (dense transformer w/ graph-masked
attention + GIN MLP). Per-core program: 2 batches, L=512, C=512, H=4, HS=128,
HID=2048. Data-parallel over batch across 8 cores, no collectives.

v8 strategy (per batch), all-bf16 matmuls + targeted fp8 DoubleRow where a
single-fp8 operand is error-safe (DR contracts 2x K per call; per-call cost
on HW is ~ap_size cycles regardless of dtype, so only call-count reduction
helps):
  - Host casts x/rel_pos/adj and all weights to bf16 (exact for
    rel_pos/adj); ln gammas folded into w_qkv/w_fc1/w_gcn rows host-side
    (betas are zero for this model). w_fc1 cast to fp8e4 host-side.
  - LN: per-chunk pipeline: bn_stats/bn_aggr (DVE) -> sqrt(var+eps) on ACT
    -> reciprocal on DVE -> xc=(x-mu)*istd (DVE, token-major bf16, kept for
    GIN u-matmuls) -> PE-transpose (4 chunks into one [P,512] bf16 PSUM)
    -> one strided copyback (DVE) -> xnT (bf16 for LN1, fp8e4 for LN2).
  - masks: tabs=|rel-5| via ACT Abs; a=is_eq(tabs,4) -> bf16 + fp8 (DVE).
    aT via PE transposes + merged copybacks. m2=aTa, m3=aaT via fp8
    DoubleRow matmuls (binary exact, K=256/call), binarized on ACT with
    Sign, diagonals set to 1 via gpsimd affine_select. Masks applied
    MULTIPLICATIVELY: attnT = exp(scoreT) * maskT on DVE.
  - attention: qT,kT channel-major bf16 (DVE copyback); v token-major bf16
    (ACT copyback); scoreT = kT_chunk.T @ qT; exp on ACT; denominators via
    ones-lhsT matmuls on masked attnT; 1/denom via DVE recip + gpsimd
    partition_broadcast, fused into attn-out copyback; proj + residual ->
    x1 bf16.
  - GIN restructured: uT = (adj@xn2c)^T, (adjT@xn2c)^T channel-major
    (lhsT = token-major xc2 bf16, rhs = adjT/adj bf16), DVE copyback; hT
    accumulates z = wgcn_half.T @ uT (bf16) + fc1 via fp8 DoubleRow
    (wfc1_8 lhsT, xn2T8 rhs; K=256/call halves fc1 calls) in one PSUM;
    relu on ACT -> hT bf16. fc2 from hT chunks + residual on copyback ->
    f32 out DMA.
"""

import sys
for _p in ("/opt/trn_rl_repo", "/root/.axon_site/_ro/trn_rl_repo"):
    if _p not in sys.path:
        sys.path.append(_p)

from contextlib import ExitStack

import concourse.bass as bass
import concourse.tile as tile
from concourse import mybir
from concourse.bass import ts
from concourse.masks import make_identity

F32 = mybir.dt.float32
BF16 = mybir.dt.bfloat16
FP8 = mybir.dt.float8e4
I32 = mybir.dt.int32
OP = mybir.AluOpType
ACT = mybir.ActivationFunctionType
DR = mybir.MatmulPerfMode.DoubleRow

P = 128
L = 512
C = 512
H = 4
HS = 128
HID = 2048
NB = 2          # batches per core
LC = L // P     # 4 token chunks
CC = C // P     # 4 channel chunks
HC = HID // P   # 16 hidden chunks
EPS = 1e-5
INV_SQRT_HS = 1.0 / (HS ** 0.5)


def build_encoder_program(nc):
    """Emit the full 2-batch encoder program into `nc`."""
    def dram(name, shape, dt, kind="ExternalInput"):
        return nc.dram_tensor(name, shape, dt, kind=kind).ap()

    x_d = dram("x", [NB, L, C], BF16)
    rp_d = dram("rel_pos", [NB, L, L], BF16)
    adj_d = dram("adj", [NB, L, L], BF16)
    wqkv_d = dram("w_qkv", [C, 3 * C], BF16)
    wproj_d = dram("w_proj", [C, C], BF16)
    wfc1_d = dram("w_fc1", [C, HID], FP8)
    wgcn_d = dram("w_gcn", [C, HID], BF16)
    wfc2_d = dram("w_fc2", [HID, C], BF16)
    out_d = dram("out", [NB, L, C], F32, kind="ExternalOutput")

    x_t3 = [x_d[b].rearrange("(lo p) c -> p lo c", p=P) for b in range(NB)]
    rp_t3 = [rp_d[b].rearrange("(lo p) c -> p lo c", p=P) for b in range(NB)]
    adj_t3 = [adj_d[b].rearrange("(lo p) c -> p lo c", p=P) for b in range(NB)]
    out_t3 = [out_d[b].rearrange("(lo p) c -> p lo c", p=P) for b in range(NB)]

    with ExitStack() as top:
        tc = top.enter_context(tile.TileContext(nc))
        const = top.enter_context(tc.tile_pool(name="const", bufs=1))
        persist = top.enter_context(tc.tile_pool(name="persist", bufs=1))
        ginpre = top.enter_context(tc.tile_pool(name="ginpre", bufs=1))
        psum = top.enter_context(tc.tile_pool(name="psum", bufs=1, space="PSUM"))
        attn_stack = ExitStack()
        wA = attn_stack.enter_context(tc.tile_pool(name="wA", bufs=1))
        ap = attn_stack.enter_context(tc.tile_pool(name="attn", bufs=1))

        def pmm():
            return psum.tile([P, 512], F32, tag="mm", bufs=4, name="pmm")

        def ptp():
            # transpose staging: 4 [P,128] bf16 transposes -> one [P,512]
            return psum.tile([P, 512], BF16, tag="tp", bufs=2, name="ptp")

        # ---- input DMAs first: order = startup critical path ----
        x_t = [ap.tile([P, LC, C], BF16, tag="x_t", bufs=2, name="x_t")
               for _ in range(NB)]
        nc.sync.dma_start(out=x_t[0][:, 0:2, :], in_=x_t3[0][:, 0:2, :])
        nc.sync.dma_start(out=x_t[0][:, 2:4, :], in_=x_t3[0][:, 2:4, :])
        rel_t = [ap.tile([P, LC, L], BF16, tag="rel", bufs=2, name="rel")
                 for _ in range(NB)]
        nc.sync.dma_start(out=rel_t[0][:], in_=rp_t3[0][:])
        wq = wA.tile([P, CC, 3 * C], BF16)
        nc.sync.dma_start(
            out=wq[:], in_=wqkv_d.rearrange("(ko p) n -> p ko n", p=P))
        wp = wA.tile([P, CC, C], BF16)
        nc.sync.dma_start(out=wp[:], in_=wproj_d.rearrange("(ko p) n -> p ko n", p=P))
        nc.sync.dma_start(out=x_t[1][:], in_=x_t3[1][:])
        nc.sync.dma_start(out=rel_t[1][:], in_=rp_t3[1][:])
        adj_b = [ginpre.tile([P, LC, L], BF16, tag="adj_b", bufs=2,
                             name="adj_b") for _ in range(NB)]
        adjT_b = [ginpre.tile([P, LC, L], BF16, tag="adjT_b", bufs=2,
                              name="adjT_b") for _ in range(NB)]
        for b in range(NB):
            nc.sync.dma_start(out=adj_b[b][:], in_=adj_t3[b][:])

        # ---------------- constants ----------------
        ident_f = const.tile([P, P], F32)
        make_identity(nc, ident_f[:])
        ident_b = const.tile([P, P], BF16)
        nc.vector.tensor_copy(out=ident_b[:], in_=ident_f[:])
        ones_b = const.tile([P, 1], BF16)
        nc.vector.memset(ones_b[:], 1.0)
        eps_t = const.tile([P, 1], F32)
        nc.vector.memset(eps_t[:], EPS)
        neg5_t = const.tile([P, 1], F32)
        nc.vector.memset(neg5_t[:], -5.0)

        # HAM warmup: dummy matmuls so the PE clock-gate opens during
        # the initial input DMAs (no data deps: memset inputs)
        warm_rhs = const.tile([P, 512], BF16)
        nc.vector.memset(warm_rhs[:], 0.0)
        warm_l = const.tile([P, P], BF16)
        nc.vector.memset(warm_l[:], 0.0)
        for _ in range(12):
            pw = pmm()
            nc.tensor.matmul(pw[:], warm_l[:], warm_rhs[:], start=True, stop=True)

        # x1 residual stream (kept across phases), bf16
        x1 = [persist.tile([P, LC, C], BF16, name=f"x1_{b}", tag=f"x1_{b}")
              for b in range(NB)]

        # ---------------- layernorm helper ----------------
        def layer_norm_T(pool, xin, tag, out_dt):
            """xin: [P, LC, C] token-major bf16. Returns (xnT, xc):
            xnT [P, CC, L] channel-major in out_dt, xc [P, LC, C]
            token-major bf16. Gamma folded into weights host-side;
            beta assumed zero. Fully per-chunk pipelined."""
            xnT = pool.tile([P, CC, L], out_dt, tag=f"xnT_{tag}", name="xnT")
            xc = pool.tile([P, LC, C], BF16, tag=f"xc_{tag}", name="xc")
            y4 = pool.tile([P, LC], F32, tag="ln_y4", bufs=2, name="y4")
            for i in range(LC):
                st6 = pool.tile([P, 6], F32, tag="ln_st6", bufs=2, name="st6")
                nc.vector.bn_stats(out=st6[:], in_=xin[:, i, :])
                mv = pool.tile([P, 2], F32, tag="ln_mv", bufs=4, name="mv")
                nc.vector.bn_aggr(out=mv[:], in_=st6[:])
                # istd = 1/sqrt(var + eps): Sqrt on ACT, recip on DVE
                sq = pool.tile([P, 1], F32, tag="ln_sq", bufs=4, name="sq")
                nc.scalar.activation(out=sq[:], in_=mv[:, 1:2],
                                     func=ACT.Sqrt, bias=eps_t[:], scale=1.0)
                nc.vector.reciprocal_approx_fast(out=y4[:, i:i + 1], in_=sq[:])
                nc.vector.tensor_scalar(out=xc[:, i, :], in0=xin[:, i, :],
                                        scalar1=mv[:, 0:1],
                                        scalar2=y4[:, i:i + 1],
                                        op0=OP.subtract, op1=OP.mult)
                pt = ptp()
                for j in range(CC):
                    nc.tensor.transpose(pt[:, ts(j, P)], xc[:, i, ts(j, P)],
                                        ident_b[:])
                nc.vector.tensor_copy(out=xnT[:, :, ts(i, P)],
                                      in_=pt[:].rearrange("p (j l) -> p j l", j=CC))
            return xnT, xc

        def set_diag1(ap_2d, m):
            """Set the diagonal-block entries of mask chunk m to 1 in place."""
            nc.gpsimd.affine_select(out=ap_2d, in_=ap_2d,
                                    compare_op=OP.not_equal, fill=1.0,
                                    base=P * m, pattern=[[-1, L]],
                                    channel_multiplier=1)

        # ---------- hop masks: m0=a|I, m1=aT|I, m2=aTa|I, m3=aaT|I ----------
        def masks_phase(b):
            a_b = ap.tile([P, LC, L], BF16, tag="a_b", bufs=2, name="a_b")
            a_8 = ap.tile([P, LC, L], FP8, tag="a_8", bufs=2, name="a_8")
            aT_b = ap.tile([P, LC, L], BF16, tag="aT_b", bufs=2, name="aT_b")
            aT_8 = ap.tile([P, LC, L], FP8, tag="aT_8", bufs=2, name="aT_8")
            m2 = ap.tile([P, LC, L], BF16, tag="m2", bufs=2, name="m2")
            m3 = ap.tile([P, LC, L], BF16, tag="m3", bufs=2, name="m3")
            for i in range(LC):
                tabs = ap.tile([P, L], BF16, tag="tabs", bufs=3, name="tabs")
                nc.scalar.activation(out=tabs[:], in_=rel_t[b][:, i, :],
                                     func=ACT.Abs, bias=neg5_t[:], scale=1.0)
                nc.vector.tensor_scalar(out=a_b[:, i, :], in0=tabs[:],
                                        scalar1=4.0, scalar2=None,
                                        op0=OP.is_equal)
                nc.vector.tensor_scalar(out=a_8[:, i, :], in0=tabs[:],
                                        scalar1=4.0, scalar2=None,
                                        op0=OP.is_equal)
            # aT via PE transposes (bf16) + merged copybacks to bf16 + fp8
            for i in range(LC):
                pt = ptp()
                for j in range(CC):
                    nc.tensor.transpose(pt[:, ts(j, P)], a_b[:, i, ts(j, P)],
                                        ident_b[:])
                src = pt[:].rearrange("p (j l) -> p j l", j=CC)
                nc.vector.tensor_copy(out=aT_b[:, :, ts(i, P)], in_=src)
                nc.vector.tensor_scalar(out=aT_8[:, :, ts(i, P)], in0=src,
                                        scalar1=0.0, scalar2=None, op0=OP.add)
            # m2 = aTa, m3 = aaT via fp8 DoubleRow (binary exact);
            # binarize on ACT with Sign (counts >= 0)
            for (cm, src) in ((m2, a_8), (m3, aT_8)):
                for m in range(LC):
                    pm = pmm()
                    for k in range(LC // 2):
                        nc.tensor.matmul(pm[:],
                                         src[:, 2 * k:2 * k + 2, ts(m, P)],
                                         src[:, 2 * k:2 * k + 2, :],
                                         start=(k == 0), stop=(k == 1),
                                         perf_mode=DR)
                    nc.scalar.activation(out=cm[:, m, :], in_=pm[:],
                                         func=ACT.Sign)
                    set_diag1(cm[:, m, :], m)
            for i in range(LC):
                set_diag1(a_b[:, i, :], i)    # a_b becomes m0 in place
                set_diag1(aT_b[:, i, :], i)   # aT_b becomes m1 in place
            # maskT per head: scoreT chunk [lk, lq] masked by pe_h[lq, lk]^T
            return [aT_b, a_b, m2, m3]

        # ================= attention =================
        def attn_ln(b):
            xn1T, _ = layer_norm_T(ap, x_t[b], "1", BF16)
            return xn1T

        def attn_phase(b, xn1T, maskT):
            # ---- qT, kT (channel-major), v (token-major), all bf16 ----
            qT = ap.tile([P, H, L], BF16, tag="qT", bufs=2, name="qT")
            kT = ap.tile([P, H, L], BF16, tag="kT", bufs=2, name="kT")
            for dst, off in ((qT, 0), (kT, C)):
                for m in range(CC):
                    pm = pmm()
                    for k in range(CC):
                        nc.tensor.matmul(pm[:], wq[:, k, off + m * P:off + (m + 1) * P],
                                         xn1T[:, k, :],
                                         start=(k == 0), stop=(k == CC - 1))
                    nc.vector.tensor_copy(out=dst[:, m, :], in_=pm[:])
            v_sb = ap.tile([P, LC, C], BF16, tag="v_sb", bufs=2, name="v_sb")
            for m in range(LC):
                pm = pmm()
                for k in range(CC):
                    nc.tensor.matmul(pm[:], xn1T[:, k, ts(m, P)],
                                     wq[:, k, 2 * C:3 * C],
                                     start=(k == 0), stop=(k == CC - 1))
                nc.scalar.copy(out=v_sb[:, m, :], in_=pm[:])

            # ---- attention heads (all 4 pipelined) ----
            OT = ap.tile([P, H, L], BF16, tag="OT", bufs=2, name="OT")
            atts = {}
            for h in range(H):
                attnT = ap.tile([P, LC, L], BF16, tag="attnT", bufs=4,
                                name="attnT")
                atts[h] = attnT
                for i in range(LC):
                    pm = pmm()
                    nc.tensor.matmul(pm[:], kT[:, h, ts(i, P)], qT[:, h, :],
                                     start=True, stop=True)
                    e_b = ap.tile([P, L], BF16, tag="e_b", bufs=6, name="e_b")
                    nc.scalar.activation(out=e_b[:], in_=pm[:],
                                         func=ACT.Exp, scale=INV_SQRT_HS)
                    nc.vector.tensor_tensor(out=attnT[:, i, :], in0=e_b[:],
                                            in1=maskT[h][:, i, :], op=OP.mult)
            rbcs = {}
            for h in range(H):
                pd = psum.tile([1, L], F32, tag="dn", bufs=1, name="pd")
                for i in range(LC):
                    nc.tensor.matmul(pd[:], ones_b[:], atts[h][:, i, :],
                                     start=(i == 0), stop=(i == LC - 1))
                recip = ap.tile([1, L], F32, tag="recip", bufs=2, name="recip")
                nc.vector.reciprocal_approx_fast(out=recip[:], in_=pd[:])
                rbc = ap.tile([P, L], F32, tag="rbc", bufs=2, name="rbc")
                nc.gpsimd.partition_broadcast(rbc[:], recip[:])
                rbcs[h] = rbc
            for h in range(H):
                po = pmm()
                for i in range(LC):
                    nc.tensor.matmul(po[:], v_sb[:, i, ts(h, P)],
                                     atts[h][:, i, :],
                                     start=(i == 0), stop=(i == LC - 1))
                nc.vector.tensor_tensor(out=OT[:, h, :], in0=po[:],
                                        in1=rbcs[h][:], op=OP.mult)

            # ---- proj + residual -> x1 (bf16) ----
            for m in range(LC):
                pm = pmm()
                for k in range(CC):
                    nc.tensor.matmul(pm[:], OT[:, k, ts(m, P)], wp[:, k, :],
                                     start=(k == 0), stop=(k == CC - 1))
                nc.vector.tensor_tensor(out=x1[b][:, m, :], in0=x_t[b][:, m, :],
                                        in1=pm[:], op=OP.add)

        # ---------- GIN prerequisites (overlap other batch's attention) ----
        def gin_pre(b):
            for i in range(LC):
                pt = ptp()
                for j in range(CC):
                    nc.tensor.transpose(pt[:, ts(j, P)], adj_b[b][:, i, ts(j, P)],
                                        ident_b[:])
                nc.vector.tensor_copy(
                    out=adjT_b[b][:, :, ts(i, P)],
                    in_=pt[:].rearrange("p (j l) -> p j l", j=CC))
            xn2T8, xc2 = layer_norm_T(ginpre, x1[b], "2", FP8)
            return xn2T8, xc2

        gin_inputs = {}
        ln0 = attn_ln(0)
        mk0 = masks_phase(0)
        attn_phase(0, ln0, mk0)
        # gin_pre(0) first: its adjT transposes are PE-ready immediately
        # (adj long DMAed), filling the boundary while DVE drains heads(0)
        gin_inputs[0] = gin_pre(0)
        ln1 = attn_ln(1)
        mk1 = masks_phase(1)
        attn_phase(1, ln1, mk1)
        gin_inputs[1] = gin_pre(1)
        attn_stack.close()

        # ================= GIN main =================
        with ExitStack() as gin_stack:
            wB = gin_stack.enter_context(tc.tile_pool(name="wB", bufs=1))
            gp = gin_stack.enter_context(tc.tile_pool(name="gin", bufs=1))

            wgc = wB.tile([P, CC, HID], BF16)
            nc.sync.dma_start(out=wgc[:],
                              in_=wgcn_d.rearrange("(ko p) n -> p ko n", p=P))
            wf1_8 = wB.tile([P, CC, HID], FP8)
            nc.sync.dma_start(out=wf1_8[:],
                              in_=wfc1_d.rearrange("(ko p) n -> p ko n", p=P))
            wf2_b = wB.tile([P, HC, C], BF16)
            wfc2_r3 = wfc2_d.rearrange("(ko p) n -> p ko n", p=P)
            nc.sync.dma_start(out=wf2_b[:, 0:8, :], in_=wfc2_r3[:, 0:8, :])
            nc.sync.dma_start(out=wf2_b[:, 8:16, :], in_=wfc2_r3[:, 8:16, :])

            for b in range(NB):
                xn2T8, xc2 = gin_inputs[b]

                # ---- uT: u1 = adj@xn2c (chunks 0-3), u2 = adjT@xn2c (4-7) ----
                uT = gp.tile([P, 2 * CC, L], BF16, tag="uT", name="uT")
                for c in range(CC):
                    pm = pmm()
                    for lk in range(LC):
                        nc.tensor.matmul(pm[:], xc2[:, lk, ts(c, P)],
                                         adjT_b[b][:, lk, :],
                                         start=(lk == 0), stop=(lk == LC - 1))
                    nc.vector.tensor_copy(out=uT[:, c, :], in_=pm[:])
                for c in range(CC):
                    pm = pmm()
                    for lk in range(LC):
                        nc.tensor.matmul(pm[:], xc2[:, lk, ts(c, P)],
                                         adj_b[b][:, lk, :],
                                         start=(lk == 0), stop=(lk == LC - 1))
                    nc.vector.tensor_copy(out=uT[:, CC + c, :], in_=pm[:])

                # ---- hT = relu(z + fc1)^T (hid-major) ----
                hT_b = gp.tile([P, HC, L], BF16, tag="hT_b", name="hT_b")
                for mh in range(HC):
                    pm = pmm()
                    uoff = 0 if mh < HC // 2 else CC
                    for k in range(CC):
                        nc.tensor.matmul(pm[:], wgc[:, k, ts(mh, P)],
                                         uT[:, uoff + k, :],
                                         start=(k == 0), stop=False)
                    for k2 in range(CC // 2):
                        nc.tensor.matmul(pm[:],
                                         wf1_8[:, 2 * k2:2 * k2 + 2, ts(mh, P)],
                                         xn2T8[:, 2 * k2:2 * k2 + 2, :],
                                         start=False, stop=(k2 == CC // 2 - 1),
                                         perf_mode=DR)
                    nc.scalar.activation(out=hT_b[:, mh, :], in_=pm[:],
                                         func=ACT.Relu)

                # ---- out = x1 + hT.T @ w_fc2 ----
                for m in range(LC):
                    pm = pmm()
                    for k in range(HC):
                        nc.tensor.matmul(pm[:], hT_b[:, k, ts(m, P)], wf2_b[:, k, :],
                                         start=(k == 0), stop=(k == HC - 1))
                    o_sb = gp.tile([P, C], F32, tag="o_sb", bufs=2, name="o_sb")
                    nc.vector.tensor_tensor(out=o_sb[:], in0=x1[b][:, m, :],
                                            in1=pm[:], op=OP.add)
                    nc.sync.dma_start(out=out_t3[b][:, m, :], in_=o_sb[:])


# ======================= SPMD wrapper =======================
import numpy as np
import ml_dtypes

N_CORES = 8
_CACHE = {}
_BF16 = ml_dtypes.bfloat16
_F8 = ml_dtypes.float8_e4m3


def _get_program():
    if "nc" not in _CACHE:
        from concourse import bacc
        nc = bacc.Bacc("TRN2", target_bir_lowering=False, debug=False,
                       num_devices=N_CORES)
        build_encoder_program(nc)
        nc.finalize()
        _CACHE["nc"] = nc
    return _CACHE["nc"]


def make_in_maps(inputs):
    """Host-side prep: fold ln gammas into weights, cast to device dtypes,
    shard the batch dim across cores."""
    f32 = lambda k: np.asarray(inputs[k], np.float32)
    g1 = f32("ln1_g")[:, None]
    g2 = f32("ln2_g")[:, None]
    shared = {
        "w_qkv": (g1 * f32("w_qkv")).astype(_BF16),
        "w_proj": f32("w_proj").astype(_BF16),
        "w_fc1": np.clip(g2 * f32("w_fc1"), -240, 240).astype(_F8),
        "w_gcn": (g2 * f32("w_gcn")).astype(_BF16),
        "w_fc2": f32("w_fc2").astype(_BF16),
    }
    x_b = np.asarray(inputs["x"], np.float32).astype(_BF16)
    rp_b = np.asarray(inputs["rel_pos"], np.float32).astype(_BF16)
    adj_bf = np.asarray(inputs["adj"], np.float32).astype(_BF16)
    in_maps = []
    for c in range(N_CORES):
        sl = slice(NB * c, NB * (c + 1))
        m = dict(shared)
        m["x"] = np.ascontiguousarray(x_b[sl])
        m["rel_pos"] = np.ascontiguousarray(rp_b[sl])
        m["adj"] = np.ascontiguousarray(adj_bf[sl])
        in_maps.append(m)
    return in_maps


def kernel(**inputs):
    """Full-input entry point: shards batch dim over 8 NeuronCores,
    runs the Bass program, gathers the full output."""
    from concourse.bass_utils import run_bass_kernel_spmd

    nc = _get_program()
    B = inputs["x"].shape[0]
    assert B == NB * N_CORES, f"expected B={NB * N_CORES}, got {B}"
    res = run_bass_kernel_spmd(nc, make_in_maps(inputs), list(range(N_CORES)))
    return np.concatenate([res.results[c]["out"] for c in range(N_CORES)], axis=0)
